# revision 5
# baseline (speedup 1.0000x reference)
"""KascadeAnchorAttention Trainium2 kernel.

Sharding: one (batch=0, head h) pair per NeuronCore (8 heads / 8 cores).
Each core computes, for its head, in fp32:
  Q^T/K^T = RoPE(w^T @ x^T)  [D=128, S=4096]   (transposed layouts, D on partitions)
  V^T then PE-transposed to V natural [S, D] blocks
  rep-tile scores  (max of rep-row logits per 128-key tile; ranking equals the
                    reference's softmax-weight ranking since softmax is monotone
                    per row)
  S^T blocks = K @ Q^T  -> exp (no row-max needed: |logits| <= ~5) -> P~^T
  denominators via ones-matmul on a DVE-accumulated running sum
  out^T = V^T @ P~^T  (PSUM accumulation over key chunks, causal-skipped)
  partial^T = wo_h^T @ (out^T * 1/denom)  -> DRAM [E, S]
Host: sums per-core partials (transposed) into out [1, S, E]; computes top-k
tile indices from the scores with jax.lax.top_k tie-break semantics
(stable argsort descending).
"""

import math

import numpy as np

B, S, H, D = 1, 4096, 8, 128
E = H * D
TILE = 128
TOPK = 8
NT = S // TILE          # 32 key tiles
QB = 512                # query block width
NQB = S // QB           # 8 query blocks
N_CORES = 8

_CACHE = {}


def _build_nc():
    from contextlib import ExitStack

    import concourse.mybir as mybir
    import concourse.tile as tile
    from concourse import bacc
    from concourse.masks import make_identity, make_upper_triangular

    f32 = mybir.dt.float32
    AF = mybir.ActivationFunctionType

    nc = bacc.Bacc(None, target_bir_lowering=False, debug=False, num_devices=N_CORES)

    xT = nc.dram_tensor("xT", [E, S], f32, kind="ExternalInput")
    wqh = nc.dram_tensor("wqh", [E, D], f32, kind="ExternalInput")
    wkh = nc.dram_tensor("wkh", [E, D], f32, kind="ExternalInput")
    wvh = nc.dram_tensor("wvh", [E, D], f32, kind="ExternalInput")
    woh = nc.dram_tensor("woh", [D, E], f32, kind="ExternalInput")
    cosT = nc.dram_tensor("cosT", [D, S], f32, kind="ExternalInput")
    sinT = nc.dram_tensor("sinT", [D, S], f32, kind="ExternalInput")
    out_part = nc.dram_tensor("out_part", [E, S], f32, kind="ExternalOutput")
    scores_o = nc.dram_tensor("scores", [NT, NT], f32, kind="ExternalOutput")

    rsqrt_d = 1.0 / math.sqrt(D)

    with tile.TileContext(nc) as tc, ExitStack() as ctx:
        const = ctx.enter_context(tc.tile_pool(name="const", bufs=1))
        big = ctx.enter_context(tc.tile_pool(name="big", bufs=1))
        xpool = ctx.enter_context(tc.tile_pool(name="xpool", bufs=2))
        rope = ctx.enter_context(tc.tile_pool(name="rope", bufs=2))
        epool = ctx.enter_context(tc.tile_pool(name="epool", bufs=4))
        apool = ctx.enter_context(tc.tile_pool(name="apool", bufs=2))
        spool = ctx.enter_context(tc.tile_pool(name="spool", bufs=2))
        pm = ctx.enter_context(tc.tile_pool(name="pm", bufs=5, space="PSUM"))
        ppo = ctx.enter_context(tc.tile_pool(name="ppo", bufs=2, space="PSUM"))
        ppt = ctx.enter_context(tc.tile_pool(name="ppt", bufs=1, space="PSUM"))

        # ---- constants ----
        ident = const.tile([128, 128], f32)
        make_identity(nc, ident)
        tri = const.tile([128, 128], f32)
        make_upper_triangular(nc, tri, val=1.0, diag=True)  # tri[k,q]=1 iff k<=q
        ones_col = const.tile([128, 1], f32)
        nc.vector.memset(ones_col, 1.0)
        ones_row = const.tile([1, 128], f32)
        nc.vector.memset(ones_row, 1.0)

        wq_sb = const.tile([128, 8, D], f32)
        wk_sb = const.tile([128, 8, D], f32)
        wv_sb = const.tile([128, 8, D], f32)
        wo_sb = const.tile([128, 8, 128], f32)
        nc.sync.dma_start(out=wq_sb, in_=wqh.ap().rearrange("(n p) d -> p n d", p=128))
        nc.sync.dma_start(out=wk_sb, in_=wkh.ap().rearrange("(n p) d -> p n d", p=128))
        nc.sync.dma_start(out=wv_sb, in_=wvh.ap().rearrange("(n p) d -> p n d", p=128))
        nc.sync.dma_start(out=wo_sb, in_=woh.ap().rearrange("d (n e) -> d n e", n=8))
        cos_sb = big.tile([128, S], f32)
        sin_sb = big.tile([128, S], f32)
        nc.sync.dma_start(out=cos_sb, in_=cosT.ap())
        nc.sync.dma_start(out=sin_sb, in_=sinT.ap())

        QT = big.tile([128, S], f32)
        KT = big.tile([128, S], f32)
        VT = big.tile([128, S], f32)
        Vn = big.tile([128, S], f32)

        # ---- Phase A: QKV projections (transposed) + RoPE ----
        for g in range(NQB):
            cols = slice(g * QB, (g + 1) * QB)
            xp = xpool.tile([128, 8, QB], f32)
            nc.sync.dma_start(
                out=xp,
                in_=xT.ap()[:, cols].rearrange("(n p) s -> p n s", p=128),
            )
            pq = pm.tile([128, QB], f32, tag="mm")
            pk = pm.tile([128, QB], f32, tag="mm")
            pv = pm.tile([128, QB], f32, tag="mm")
            for eb in range(8):
                st, sp = eb == 0, eb == 7
                nc.tensor.matmul(pq, lhsT=wq_sb[:, eb, :], rhs=xp[:, eb, :], start=st, stop=sp)
                nc.tensor.matmul(pk, lhsT=wk_sb[:, eb, :], rhs=xp[:, eb, :], start=st, stop=sp)
                nc.tensor.matmul(pv, lhsT=wv_sb[:, eb, :], rhs=xp[:, eb, :], start=st, stop=sp)
            # RoPE: out[0:64] = p[0:64]*cos[0:64] - p[64:128]*sin[0:64]
            #       out[64:128] = p[64:128]*cos[64:128] + p[0:64]*sin[64:128]
            for src, dst in ((pq, QT), (pk, KT)):
                tc_full = rope.tile([128, QB], f32, tag="ropec")
                rot = rope.tile([128, QB], f32, tag="roper")
                nc.vector.tensor_copy(rot[0:64, :], src[64:128, :])
                nc.vector.tensor_copy(rot[64:128, :], src[0:64, :])
                nc.vector.tensor_mul(rot, rot, sin_sb[:, cols])
                nc.vector.tensor_mul(tc_full, src, cos_sb[:, cols])
                nc.vector.tensor_sub(dst[0:64, cols], tc_full[0:64, :], rot[0:64, :])
                nc.vector.tensor_add(dst[64:128, cols], tc_full[64:128, :], rot[64:128, :])
            nc.vector.tensor_copy(VT[:, cols], pv)

        # ---- Phase A2: V natural blocks via PE transpose ----
        for t in range(NT):
            bl = slice(t * 128, (t + 1) * 128)
            pt = ppt.tile([128, 128], f32, tag="pt")
            nc.tensor.transpose(pt, VT[:, bl], ident)
            nc.vector.tensor_copy(Vn[:, bl], pt)

        # ---- Phase B: rep-row tile scores ----
        qrep = const.tile([128, NT, 1], f32)
        nc.vector.tensor_copy(
            qrep, QT.rearrange("d (t c) -> d t c", c=128)[:, :, 127:128]
        )
        qrep2 = qrep.rearrange("d t one -> d (t one)")
        scores_sb = const.tile([NT, NT], f32)
        for kb in range(8):
            pr = pm.tile([NT, QB], f32, tag="mm")
            nc.tensor.matmul(
                pr, lhsT=qrep2, rhs=KT[:, kb * QB : (kb + 1) * QB], start=True, stop=True
            )
            nc.vector.tensor_reduce(
                out=scores_sb[:, kb * 4 : (kb + 1) * 4],
                in_=pr.rearrange("p (a b) -> p a b", b=128),
                axis=mybir.AxisListType.X,
                op=mybir.AluOpType.max,
            )
        nc.sync.dma_start(out=scores_o.ap(), in_=scores_sb)

        # ---- Phase C: causal attention + output projection ----
        for g in range(NQB):
            cols = slice(g * QB, (g + 1) * QB)
            nkb = 4 * g + 4
            po = ppo.tile([128, QB], f32, tag="po")
            A_acc = apool.tile([128, QB], f32, tag="acc")
            for skb in range(nkb):
                bl = slice(skb * 128, (skb + 1) * 128)
                ps = pm.tile([128, QB], f32, tag="mm")
                nc.tensor.matmul(ps, lhsT=KT[:, bl], rhs=QT[:, cols], start=True, stop=True)
                Et = epool.tile([128, QB], f32, tag="et")
                nc.scalar.activation(out=Et, in_=ps, func=AF.Exp, scale=rsqrt_d)
                r = skb - 4 * g
                if r >= 0:
                    if r > 0:
                        nc.vector.memset(Et[:, 0 : r * 128], 0.0)
                    dg = slice(r * 128, (r + 1) * 128)
                    nc.vector.tensor_mul(Et[:, dg], Et[:, dg], tri)
                if skb == 0:
                    nc.vector.tensor_copy(A_acc, Et)
                else:
                    nc.vector.tensor_add(A_acc, A_acc, Et)
                nc.tensor.matmul(
                    po, lhsT=Vn[:, bl], rhs=Et, start=(skb == 0), stop=(skb == nkb - 1)
                )
            pdn = pm.tile([1, QB], f32, tag="mm")
            nc.tensor.matmul(pdn, lhsT=ones_col, rhs=A_acc, start=True, stop=True)
            recip = spool.tile([1, QB], f32, tag="rec")
            nc.vector.reciprocal(recip, pdn)
            pb = pm.tile([128, QB], f32, tag="mm")
            nc.tensor.matmul(pb, lhsT=ones_row, rhs=recip, start=True, stop=True)
            rb = spool.tile([128, QB], f32, tag="rb")
            nc.vector.tensor_copy(rb, pb)
            onorm = apool.tile([128, QB], f32, tag="onorm")
            nc.vector.tensor_mul(onorm, po, rb)
            for eb in range(8):
                pw = pm.tile([128, QB], f32, tag="mm")
                nc.tensor.matmul(pw, lhsT=wo_sb[:, eb, :], rhs=onorm, start=True, stop=True)
                wout = epool.tile([128, QB], f32, tag="wout")
                nc.vector.tensor_copy(wout, pw)
                nc.sync.dma_start(
                    out=out_part.ap()[eb * 128 : (eb + 1) * 128, cols], in_=wout
                )

    nc.finalize()
    return nc


def kernel(x, wq, wk, wv, wo, cos, sin):
    from concourse.bass_utils import run_bass_kernel_spmd

    if "nc" not in _CACHE:
        _CACHE["nc"] = _build_nc()
    nc = _CACHE["nc"]

    xT = np.ascontiguousarray(x[0].T.astype(np.float32))
    cosT = np.ascontiguousarray(np.concatenate([cos.T, cos.T], axis=0).astype(np.float32))
    sinT = np.ascontiguousarray(np.concatenate([sin.T, sin.T], axis=0).astype(np.float32))
    in_maps = []
    for h in range(N_CORES):
        hs = slice(h * D, (h + 1) * D)
        in_maps.append(
            {
                "xT": xT,
                "wqh": np.ascontiguousarray(wq[:, hs]),
                "wkh": np.ascontiguousarray(wk[:, hs]),
                "wvh": np.ascontiguousarray(wv[:, hs]),
                "woh": np.ascontiguousarray(wo[hs, :]),
                "cosT": cosT,
                "sinT": sinT,
            }
        )

    res = run_bass_kernel_spmd(
        nc, in_maps, core_ids=list(range(N_CORES)), trace=_CACHE.get("trace", False)
    )
    _CACHE["last_res"] = res

    out = np.zeros((S, E), dtype=np.float32)
    idxs = []
    for c in range(N_CORES):
        out += res.results[c]["out_part"].T
        sc = res.results[c]["scores"].copy()
        ti = np.arange(NT)
        sc[ti[:, None] < ti[None, :]] = -np.inf  # causal: key tile j > query tile t
        order = np.argsort(-sc, axis=-1, kind="stable")[:, :TOPK]
        idxs.append(order.astype(np.int32))
    return out[None], np.stack(idxs)[None]


# revision 8
# speedup vs baseline: 1.0949x; 1.0949x over previous
"""KascadeAnchorAttention Trainium2 kernel.

Sharding: one (batch=0, head h) pair per NeuronCore (8 heads / 8 cores).
Each core computes, for its head, in fp32:
  Q^T/K^T = RoPE(w^T @ x^T)  [D=128, S=4096]   (transposed layouts, D on partitions)
  V^T then PE-transposed to V natural [S, D] blocks
  rep-tile scores  (max of rep-row logits per 128-key tile; ranking equals the
                    reference's softmax-weight ranking since softmax is monotone
                    per row)
  S^T blocks = K @ Q^T  -> exp (no row-max needed: |logits| <= ~5) -> P~^T
  denominators via ones-matmul on a DVE-accumulated running sum
  out^T = V^T @ P~^T  (PSUM accumulation over key chunks, causal-skipped)
  partial^T = wo_h^T @ (out^T * 1/denom)  -> DRAM [E, S]
Host: sums per-core partials (transposed) into out [1, S, E]; computes top-k
tile indices from the scores with jax.lax.top_k tie-break semantics
(stable argsort descending).
"""

import math

import numpy as np

B, S, H, D = 1, 4096, 8, 128
E = H * D
TILE = 128
TOPK = 8
NT = S // TILE          # 32 key tiles
QB = 512                # query block width
NQB = S // QB           # 8 query blocks
N_CORES = 8

_CACHE = {}


def _build_nc():
    from contextlib import ExitStack

    import concourse.mybir as mybir
    import concourse.tile as tile
    from concourse import bacc
    from concourse.masks import make_identity, make_upper_triangular

    f32 = mybir.dt.float32
    AF = mybir.ActivationFunctionType

    nc = bacc.Bacc(None, target_bir_lowering=False, debug=False, num_devices=N_CORES)

    xT = nc.dram_tensor("xT", [E, S], f32, kind="ExternalInput")
    wqh = nc.dram_tensor("wqh", [E, D], f32, kind="ExternalInput")
    wkh = nc.dram_tensor("wkh", [E, D], f32, kind="ExternalInput")
    wvh = nc.dram_tensor("wvh", [E, D], f32, kind="ExternalInput")
    woh = nc.dram_tensor("woh", [D, E], f32, kind="ExternalInput")
    cosT = nc.dram_tensor("cosT", [D, S], f32, kind="ExternalInput")
    sinT = nc.dram_tensor("sinT", [D, S], f32, kind="ExternalInput")
    out_part = nc.dram_tensor("out_part", [E, S], f32, kind="ExternalOutput")
    scores_o = nc.dram_tensor("scores", [NT, NT], f32, kind="ExternalOutput")

    rsqrt_d = 1.0 / math.sqrt(D)

    with tile.TileContext(nc) as tc, ExitStack() as ctx:
        const = ctx.enter_context(tc.tile_pool(name="const", bufs=1))
        big = ctx.enter_context(tc.tile_pool(name="big", bufs=1))
        xpool = ctx.enter_context(tc.tile_pool(name="xpool", bufs=2))
        rope = ctx.enter_context(tc.tile_pool(name="rope", bufs=2))
        epool = ctx.enter_context(tc.tile_pool(name="epool", bufs=4))
        apool = ctx.enter_context(tc.tile_pool(name="apool", bufs=2))
        spool = ctx.enter_context(tc.tile_pool(name="spool", bufs=2))
        pm = ctx.enter_context(tc.tile_pool(name="pm", bufs=5, space="PSUM"))
        ppo = ctx.enter_context(tc.tile_pool(name="ppo", bufs=2, space="PSUM"))
        ppt = ctx.enter_context(tc.tile_pool(name="ppt", bufs=1, space="PSUM"))

        # ---- constants ----
        ident = const.tile([128, 128], f32)
        make_identity(nc, ident)
        tri = const.tile([128, 128], f32)
        make_upper_triangular(nc, tri, val=1.0, diag=True)  # tri[k,q]=1 iff k<=q
        ones_col = const.tile([128, 1], f32)
        nc.vector.memset(ones_col, 1.0)
        ones_row = const.tile([1, 128], f32)
        nc.vector.memset(ones_row, 1.0)

        wq_sb = const.tile([128, 8, D], f32)
        wk_sb = const.tile([128, 8, D], f32)
        wv_sb = const.tile([128, 8, D], f32)
        wo_sb = const.tile([128, 8, 128], f32)
        nc.sync.dma_start(out=wq_sb, in_=wqh.ap().rearrange("(n p) d -> p n d", p=128))
        nc.sync.dma_start(out=wk_sb, in_=wkh.ap().rearrange("(n p) d -> p n d", p=128))
        nc.sync.dma_start(out=wv_sb, in_=wvh.ap().rearrange("(n p) d -> p n d", p=128))
        nc.sync.dma_start(out=wo_sb, in_=woh.ap().rearrange("d (n e) -> d n e", n=8))
        cos_sb = big.tile([128, S], f32)
        sin_sb = big.tile([128, S], f32)
        nc.sync.dma_start(out=cos_sb, in_=cosT.ap())
        nc.sync.dma_start(out=sin_sb, in_=sinT.ap())

        QT = big.tile([128, S], f32)
        KT = big.tile([128, S], f32)
        VT = big.tile([128, S], f32)
        Vn = big.tile([128, S], f32)

        # ---- Fused pipeline: per query-block g, project panel g (QKV+RoPE),
        # transpose its V chunks, then run causal attention for block g.
        # Attention block g only needs K/V chunks 0..4g+3 == panels 0..g,
        # so the 16MB xT stream overlaps with attention compute.
        for g in range(NQB):
            cols = slice(g * QB, (g + 1) * QB)
            xp = xpool.tile([128, 8, QB], f32)
            nc.sync.dma_start(
                out=xp,
                in_=xT.ap()[:, cols].rearrange("(n p) s -> p n s", p=128),
            )
            pq = pm.tile([128, QB], f32, tag="mm")
            pk = pm.tile([128, QB], f32, tag="mm")
            pv = pm.tile([128, QB], f32, tag="mm")
            for eb in range(8):
                st, sp = eb == 0, eb == 7
                nc.tensor.matmul(pq, lhsT=wq_sb[:, eb, :], rhs=xp[:, eb, :], start=st, stop=sp)
                nc.tensor.matmul(pk, lhsT=wk_sb[:, eb, :], rhs=xp[:, eb, :], start=st, stop=sp)
                nc.tensor.matmul(pv, lhsT=wv_sb[:, eb, :], rhs=xp[:, eb, :], start=st, stop=sp)
            # RoPE: out[0:64] = p[0:64]*cos[0:64] - p[64:128]*sin[0:64]
            #       out[64:128] = p[64:128]*cos[64:128] + p[0:64]*sin[64:128]
            for src, dst in ((pq, QT), (pk, KT)):
                tc_full = rope.tile([128, QB], f32, tag="ropec")
                rot = rope.tile([128, QB], f32, tag="roper")
                nc.vector.tensor_copy(rot[0:64, :], src[64:128, :])
                nc.vector.tensor_copy(rot[64:128, :], src[0:64, :])
                nc.vector.tensor_mul(rot, rot, sin_sb[:, cols])
                nc.vector.tensor_mul(tc_full, src, cos_sb[:, cols])
                nc.vector.tensor_sub(dst[0:64, cols], tc_full[0:64, :], rot[0:64, :])
                nc.vector.tensor_add(dst[64:128, cols], tc_full[64:128, :], rot[64:128, :])
            nc.vector.tensor_copy(VT[:, cols], pv)

            # V natural blocks for this panel via PE transpose
            for t in range(4 * g, 4 * g + 4):
                bl = slice(t * 128, (t + 1) * 128)
                pt = ppt.tile([128, 128], f32, tag="pt")
                nc.tensor.transpose(pt, VT[:, bl], ident)
                nc.vector.tensor_copy(Vn[:, bl], pt)

            # causal attention + output projection for query block g
            nkb = 4 * g + 4
            po = ppo.tile([128, QB], f32, tag="po")
            A_acc = apool.tile([128, QB], f32, tag="acc")
            for skb in range(nkb):
                bl = slice(skb * 128, (skb + 1) * 128)
                ps = pm.tile([128, QB], f32, tag="mm")
                nc.tensor.matmul(ps, lhsT=KT[:, bl], rhs=QT[:, cols], start=True, stop=True)
                Et = epool.tile([128, QB], f32, tag="et")
                nc.scalar.activation(out=Et, in_=ps, func=AF.Exp, scale=rsqrt_d)
                r = skb - 4 * g
                if r >= 0:
                    if r > 0:
                        nc.vector.memset(Et[:, 0 : r * 128], 0.0)
                    dg = slice(r * 128, (r + 1) * 128)
                    nc.vector.tensor_mul(Et[:, dg], Et[:, dg], tri)
                if skb == 0:
                    nc.vector.tensor_copy(A_acc, Et)
                else:
                    nc.vector.tensor_add(A_acc, A_acc, Et)
                nc.tensor.matmul(
                    po, lhsT=Vn[:, bl], rhs=Et, start=(skb == 0), stop=(skb == nkb - 1)
                )
            pdn = pm.tile([1, QB], f32, tag="mm")
            nc.tensor.matmul(pdn, lhsT=ones_col, rhs=A_acc, start=True, stop=True)
            recip = spool.tile([1, QB], f32, tag="rec")
            nc.vector.reciprocal(recip, pdn)
            pb = pm.tile([128, QB], f32, tag="mm")
            nc.tensor.matmul(pb, lhsT=ones_row, rhs=recip, start=True, stop=True)
            rb = spool.tile([128, QB], f32, tag="rb")
            nc.vector.tensor_copy(rb, pb)
            onorm = apool.tile([128, QB], f32, tag="onorm")
            nc.vector.tensor_mul(onorm, po, rb)
            for eb in range(8):
                pw = pm.tile([128, QB], f32, tag="mm")
                nc.tensor.matmul(pw, lhsT=wo_sb[:, eb, :], rhs=onorm, start=True, stop=True)
                wout = epool.tile([128, QB], f32, tag="wout")
                nc.vector.tensor_copy(wout, pw)
                nc.sync.dma_start(
                    out=out_part.ap()[eb * 128 : (eb + 1) * 128, cols], in_=wout
                )

        # ---- rep-row tile scores (needs full QT/KT; tiny) ----
        qrep = const.tile([128, NT, 1], f32)
        nc.vector.tensor_copy(
            qrep, QT.rearrange("d (t c) -> d t c", c=128)[:, :, 127:128]
        )
        qrep2 = qrep.rearrange("d t one -> d (t one)")
        scores_sb = const.tile([NT, NT], f32)
        for kb in range(8):
            pr = pm.tile([NT, QB], f32, tag="mm")
            nc.tensor.matmul(
                pr, lhsT=qrep2, rhs=KT[:, kb * QB : (kb + 1) * QB], start=True, stop=True
            )
            nc.vector.tensor_reduce(
                out=scores_sb[:, kb * 4 : (kb + 1) * 4],
                in_=pr.rearrange("p (a b) -> p a b", b=128),
                axis=mybir.AxisListType.X,
                op=mybir.AluOpType.max,
            )
        nc.sync.dma_start(out=scores_o.ap(), in_=scores_sb)

    nc.finalize()
    return nc


def kernel(x, wq, wk, wv, wo, cos, sin):
    from concourse.bass_utils import run_bass_kernel_spmd

    if "nc" not in _CACHE:
        _CACHE["nc"] = _build_nc()
    nc = _CACHE["nc"]

    xT = np.ascontiguousarray(x[0].T.astype(np.float32))
    cosT = np.ascontiguousarray(np.concatenate([cos.T, cos.T], axis=0).astype(np.float32))
    sinT = np.ascontiguousarray(np.concatenate([sin.T, sin.T], axis=0).astype(np.float32))
    in_maps = []
    for h in range(N_CORES):
        hs = slice(h * D, (h + 1) * D)
        in_maps.append(
            {
                "xT": xT,
                "wqh": np.ascontiguousarray(wq[:, hs]),
                "wkh": np.ascontiguousarray(wk[:, hs]),
                "wvh": np.ascontiguousarray(wv[:, hs]),
                "woh": np.ascontiguousarray(wo[hs, :]),
                "cosT": cosT,
                "sinT": sinT,
            }
        )

    res = run_bass_kernel_spmd(
        nc, in_maps, core_ids=list(range(N_CORES)), trace=_CACHE.get("trace", False)
    )
    _CACHE["last_res"] = res

    out = np.zeros((S, E), dtype=np.float32)
    idxs = []
    for c in range(N_CORES):
        out += res.results[c]["out_part"].T
        sc = res.results[c]["scores"].copy()
        ti = np.arange(NT)
        sc[ti[:, None] < ti[None, :]] = -np.inf  # causal: key tile j > query tile t
        order = np.argsort(-sc, axis=-1, kind="stable")[:, :TOPK]
        idxs.append(order.astype(np.int32))
    return out[None], np.stack(idxs)[None]


# revision 10
# speedup vs baseline: 1.1150x; 1.0183x over previous
"""KascadeAnchorAttention Trainium2 kernel.

Sharding: one (batch=0, head h) pair per NeuronCore (8 heads / 8 cores).
Each core computes, for its head, in fp32:
  Q^T/K^T = RoPE(w^T @ x^T)  [D=128, S=4096]   (transposed layouts, D on partitions)
  V^T then PE-transposed to V natural [S, D] blocks
  rep-tile scores  (max of rep-row logits per 128-key tile; ranking equals the
                    reference's softmax-weight ranking since softmax is monotone
                    per row)
  S^T blocks = K @ Q^T  -> exp (no row-max needed: |logits| <= ~5) -> P~^T
  denominators via ones-matmul on a DVE-accumulated running sum
  out^T = V^T @ P~^T  (PSUM accumulation over key chunks, causal-skipped)
  partial^T = wo_h^T @ (out^T * 1/denom)  -> DRAM [E, S]
Host: sums per-core partials (transposed) into out [1, S, E]; computes top-k
tile indices from the scores with jax.lax.top_k tie-break semantics
(stable argsort descending).
"""

import math

import numpy as np

B, S, H, D = 1, 4096, 8, 128
E = H * D
TILE = 128
TOPK = 8
NT = S // TILE          # 32 key tiles
QB = 512                # query block width
NQB = S // QB           # 8 query blocks
N_CORES = 8

_CACHE = {}


def _build_nc():
    from contextlib import ExitStack

    import concourse.mybir as mybir
    import concourse.tile as tile
    from concourse import bacc
    from concourse.masks import make_identity, make_upper_triangular

    f32 = mybir.dt.float32
    AF = mybir.ActivationFunctionType

    nc = bacc.Bacc(None, target_bir_lowering=False, debug=False, num_devices=N_CORES)

    xT = nc.dram_tensor("xT", [E, S], f32, kind="ExternalInput")
    wqh = nc.dram_tensor("wqh", [E, D], f32, kind="ExternalInput")
    wkh = nc.dram_tensor("wkh", [E, D], f32, kind="ExternalInput")
    wvh = nc.dram_tensor("wvh", [E, D], f32, kind="ExternalInput")
    woh = nc.dram_tensor("woh", [D, E], f32, kind="ExternalInput")
    cosT = nc.dram_tensor("cosT", [D, S], f32, kind="ExternalInput")
    sinT = nc.dram_tensor("sinT", [D, S], f32, kind="ExternalInput")
    out_part = nc.dram_tensor("out_part", [E, S], f32, kind="ExternalOutput")
    scores_o = nc.dram_tensor("scores", [NT, NT], f32, kind="ExternalOutput")

    rsqrt_d = 1.0 / math.sqrt(D)

    with tile.TileContext(nc) as tc, ExitStack() as ctx:
        const = ctx.enter_context(tc.tile_pool(name="const", bufs=1))
        big = ctx.enter_context(tc.tile_pool(name="big", bufs=1))
        xpool = ctx.enter_context(tc.tile_pool(name="xpool", bufs=2))
        rope = ctx.enter_context(tc.tile_pool(name="rope", bufs=2))
        epool = ctx.enter_context(tc.tile_pool(name="epool", bufs=4))
        apool = ctx.enter_context(tc.tile_pool(name="apool", bufs=2))
        spool = ctx.enter_context(tc.tile_pool(name="spool", bufs=2))
        pm = ctx.enter_context(tc.tile_pool(name="pm", bufs=4, space="PSUM"))
        ppo = ctx.enter_context(tc.tile_pool(name="ppo", bufs=2, space="PSUM"))
        ppt = ctx.enter_context(tc.tile_pool(name="ppt", bufs=1, space="PSUM"))

        # ---- constants ----
        ident = const.tile([128, 128], f32)
        make_identity(nc, ident)
        tri = const.tile([128, 128], f32)
        make_upper_triangular(nc, tri, val=1.0, diag=True)  # tri[k,q]=1 iff k<=q
        ones_col = const.tile([128, 1], f32)
        nc.vector.memset(ones_col, 1.0)
        ones_row = const.tile([1, 128], f32)
        nc.vector.memset(ones_row, 1.0)

        wq_sb = const.tile([128, 8, D], f32)
        wk_sb = const.tile([128, 8, D], f32)
        wv_sb = const.tile([128, 8, D], f32)
        wo_sb = const.tile([128, 8, 128], f32)
        nc.sync.dma_start(out=wq_sb, in_=wqh.ap().rearrange("(n p) d -> p n d", p=128))
        nc.sync.dma_start(out=wk_sb, in_=wkh.ap().rearrange("(n p) d -> p n d", p=128))
        nc.sync.dma_start(out=wv_sb, in_=wvh.ap().rearrange("(n p) d -> p n d", p=128))
        nc.sync.dma_start(out=wo_sb, in_=woh.ap().rearrange("d (n e) -> d n e", n=8))
        cos_sb = big.tile([128, S], f32)
        sin_sb = big.tile([128, S], f32)
        nc.sync.dma_start(out=cos_sb, in_=cosT.ap())
        nc.sync.dma_start(out=sin_sb, in_=sinT.ap())

        QT = big.tile([128, S], f32)
        KT = big.tile([128, S], f32)
        VT = big.tile([128, S], f32)
        Vn = big.tile([128, S], f32)

        # ---- Fused pipeline: per query-block g, project panel g (QKV+RoPE),
        # transpose its V chunks, then run causal attention for block g.
        # Attention block g only needs K/V chunks 0..4g+3 == panels 0..g,
        # so the 16MB xT stream overlaps with attention compute.
        for g in range(NQB):
            cols = slice(g * QB, (g + 1) * QB)
            xp = xpool.tile([128, 8, QB], f32)
            nc.sync.dma_start(
                out=xp,
                in_=xT.ap()[:, cols].rearrange("(n p) s -> p n s", p=128),
            )
            pq = pm.tile([128, QB], f32, tag="mm")
            pk = pm.tile([128, QB], f32, tag="mm")
            pv = pm.tile([128, QB], f32, tag="mm")
            for eb in range(8):
                st, sp = eb == 0, eb == 7
                nc.tensor.matmul(pq, lhsT=wq_sb[:, eb, :], rhs=xp[:, eb, :], start=st, stop=sp)
                nc.tensor.matmul(pk, lhsT=wk_sb[:, eb, :], rhs=xp[:, eb, :], start=st, stop=sp)
                nc.tensor.matmul(pv, lhsT=wv_sb[:, eb, :], rhs=xp[:, eb, :], start=st, stop=sp)
            # RoPE: out[0:64] = p[0:64]*cos[0:64] - p[64:128]*sin[0:64]
            #       out[64:128] = p[64:128]*cos[64:128] + p[0:64]*sin[64:128]
            for src, dst in ((pq, QT), (pk, KT)):
                tc_full = rope.tile([128, QB], f32, tag="ropec")
                rot = rope.tile([128, QB], f32, tag="roper")
                nc.vector.tensor_copy(rot[0:64, :], src[64:128, :])
                nc.vector.tensor_copy(rot[64:128, :], src[0:64, :])
                nc.vector.tensor_mul(rot, rot, sin_sb[:, cols])
                nc.vector.tensor_mul(tc_full, src, cos_sb[:, cols])
                nc.vector.tensor_sub(dst[0:64, cols], tc_full[0:64, :], rot[0:64, :])
                nc.vector.tensor_add(dst[64:128, cols], tc_full[64:128, :], rot[64:128, :])
            nc.vector.tensor_copy(VT[:, cols], pv)

            # V natural blocks for this panel via PE transpose
            for t in range(4 * g, 4 * g + 4):
                bl = slice(t * 128, (t + 1) * 128)
                pt = ppt.tile([128, 128], f32, tag="pt")
                nc.tensor.transpose(pt, VT[:, bl], ident)
                nc.vector.tensor_copy(Vn[:, bl], pt)

            # causal attention + output projection for query block g
            nkb = 4 * g + 4
            po = ppo.tile([128, QB], f32, tag="po")
            pdn = ppt.tile([1, QB], f32, tag="pd")
            for skb in range(nkb):
                bl = slice(skb * 128, (skb + 1) * 128)
                ps = pm.tile([128, QB], f32, tag="mm")
                nc.tensor.matmul(ps, lhsT=KT[:, bl], rhs=QT[:, cols], start=True, stop=True)
                Et = epool.tile([128, QB], f32, tag="et")
                nc.scalar.activation(out=Et, in_=ps, func=AF.Exp, scale=rsqrt_d)
                r = skb - 4 * g
                if r >= 0:
                    if r > 0:
                        nc.vector.memset(Et[:, 0 : r * 128], 0.0)
                    dg = slice(r * 128, (r + 1) * 128)
                    nc.vector.tensor_mul(Et[:, dg], Et[:, dg], tri)
                st, sp = skb == 0, skb == nkb - 1
                nc.tensor.matmul(po, lhsT=Vn[:, bl], rhs=Et, start=st, stop=sp)
                nc.tensor.matmul(pdn, lhsT=ones_col, rhs=Et, start=st, stop=sp)
            recip = spool.tile([1, QB], f32, tag="rec")
            nc.vector.reciprocal(recip, pdn)
            pb = pm.tile([128, QB], f32, tag="mm")
            nc.tensor.matmul(pb, lhsT=ones_row, rhs=recip, start=True, stop=True)
            rb = spool.tile([128, QB], f32, tag="rb")
            nc.vector.tensor_copy(rb, pb)
            onorm = apool.tile([128, QB], f32, tag="onorm")
            nc.vector.tensor_mul(onorm, po, rb)
            for eb in range(8):
                pw = pm.tile([128, QB], f32, tag="mm")
                nc.tensor.matmul(pw, lhsT=wo_sb[:, eb, :], rhs=onorm, start=True, stop=True)
                wout = epool.tile([128, QB], f32, tag="wout")
                nc.vector.tensor_copy(wout, pw)
                nc.sync.dma_start(
                    out=out_part.ap()[eb * 128 : (eb + 1) * 128, cols], in_=wout
                )

        # ---- rep-row tile scores (needs full QT/KT; tiny) ----
        qrep = const.tile([128, NT, 1], f32)
        nc.vector.tensor_copy(
            qrep, QT.rearrange("d (t c) -> d t c", c=128)[:, :, 127:128]
        )
        qrep2 = qrep.rearrange("d t one -> d (t one)")
        scores_sb = const.tile([NT, NT], f32)
        for kb in range(8):
            pr = pm.tile([NT, QB], f32, tag="mm")
            nc.tensor.matmul(
                pr, lhsT=qrep2, rhs=KT[:, kb * QB : (kb + 1) * QB], start=True, stop=True
            )
            nc.vector.tensor_reduce(
                out=scores_sb[:, kb * 4 : (kb + 1) * 4],
                in_=pr.rearrange("p (a b) -> p a b", b=128),
                axis=mybir.AxisListType.X,
                op=mybir.AluOpType.max,
            )
        nc.sync.dma_start(out=scores_o.ap(), in_=scores_sb)

    nc.finalize()
    return nc


def kernel(x, wq, wk, wv, wo, cos, sin):
    from concourse.bass_utils import run_bass_kernel_spmd

    if "nc" not in _CACHE:
        _CACHE["nc"] = _build_nc()
    nc = _CACHE["nc"]

    xT = np.ascontiguousarray(x[0].T.astype(np.float32))
    cosT = np.ascontiguousarray(np.concatenate([cos.T, cos.T], axis=0).astype(np.float32))
    sinT = np.ascontiguousarray(np.concatenate([sin.T, sin.T], axis=0).astype(np.float32))
    in_maps = []
    for h in range(N_CORES):
        hs = slice(h * D, (h + 1) * D)
        in_maps.append(
            {
                "xT": xT,
                "wqh": np.ascontiguousarray(wq[:, hs]),
                "wkh": np.ascontiguousarray(wk[:, hs]),
                "wvh": np.ascontiguousarray(wv[:, hs]),
                "woh": np.ascontiguousarray(wo[hs, :]),
                "cosT": cosT,
                "sinT": sinT,
            }
        )

    res = run_bass_kernel_spmd(
        nc, in_maps, core_ids=list(range(N_CORES)), trace=_CACHE.get("trace", False)
    )
    _CACHE["last_res"] = res

    out = np.zeros((S, E), dtype=np.float32)
    idxs = []
    for c in range(N_CORES):
        out += res.results[c]["out_part"].T
        sc = res.results[c]["scores"].copy()
        ti = np.arange(NT)
        sc[ti[:, None] < ti[None, :]] = -np.inf  # causal: key tile j > query tile t
        order = np.argsort(-sc, axis=-1, kind="stable")[:, :TOPK]
        idxs.append(order.astype(np.int32))
    return out[None], np.stack(idxs)[None]
